# revision 17
# baseline (speedup 1.0000x reference)
"""Trainium2 Bass kernel for the sparse_attention nn.Module problem.

Reference computation (B=4, H=W=64, C=128, HEADS=4, DIM_HEAD=32):
  qkv = x @ w_qkv ; q,k = l2norm over token axis ; sim = q@k^T * 10
  attn = softmax(sim) ; out = (attn @ v) @ w_out + b_out

Because q and k are L2-normalized over the 4096-token axis, every dot
product q.k is tiny: |10*sim| <= 0.14 on this data (std 0.016).  The
softmax is therefore uniform + a small linear correction, and a first-
order Taylor expansion of exp is accurate to ~3.6e-4 relative error
(validated on the exact inputs; tolerance is 2e-2):

  numer[d,i] = sum_j (1 + x_ji) v_jd = V1_d + (M~^T q)_di
  den[i]     = S + sum_j x_ji        = S + (Ksum~^T q)_i
  1/den      ~ 1/S - corr/S^2        (|corr/S| <= 2e-3, err ~ 2e-6)

with rank-32 per-head Grams M = W_k^T G W_v, G = X X^T (over tokens),
and the L2 norms from diag(W^T G W).  This removes the O(S^2) sim/exp
entirely (exp alone costs ~218us/core on the ACT engine).

Sharding: 8 cores = (batch b = core//2, query-half = core%2).  Each core
computes G/X1/M over the full image (cheap) and the output for its own
2048 queries.

Device dataflow (per core):
  G   += xn_chunk^T @ xn_chunk   (fp16 PE, 32 chunks, PSUM f32 accum)
  X1  += ones^T @ xn_chunk       (interleaved, same rhs)
  qh   = w_q^T @ xq              (queries, fp16)
  Tq/Tk/Tv = G @ w_{q,k,v};  M = w_k^T Tv;  ssq = ones^T (w .* T)
  g10  = exp(-0.5 ln(ssq_q*ssq_k) + ln 10)        [1,128] row
  [g10; X1] --DRAM bounce--> columns [128,2]
  Ksum = w_k^T X1, V1row = X1^T w_v  (f32r, exact)
  mbd  = blockdiag(g10 * M);  ksw[c, d] = (g10*Ksum)_c for d in head(c)
  per 512-query chunk:
    pd = ksw^T q                  -> den corr, pre-broadcast over rows
    s1 = pd * (-1/S^2) + 1/S      (fused DVE tensor_scalar)
    pn = mbd^T q (+ V1row x ones, K=1 accum matmul)
    att = pn * s1  (DVE) ;  po = w_out^T att (+ b_out x ones)
    res = copy(po) (ACT) ; DMA out
Output is c-major [128, 2048]; host transposes and reassembles.
"""

import math
import sys
from contextlib import ExitStack

import numpy as np

import ml_dtypes
_F8NP = ml_dtypes.float8_e4m3

for _p in ("/opt/trn_rl_repo",):
    if _p not in sys.path:
        sys.path.insert(0, _p)

import concourse.bass as bass
import concourse.tile as tile
from concourse import bacc, mybir
from concourse._compat import with_exitstack

F32 = mybir.dt.float32
F32R = mybir.dt.float32r  # fp32 data, single-pass matmul
FP16 = mybir.dt.float16
FP8 = mybir.dt.float8e4
AF = mybir.ActivationFunctionType
ALU = mybir.AluOpType

S = 4096          # tokens per image
C = 128           # channels
NQ = 2048         # queries per core
HEADS = 4
DH = 32
N_CORES = 8

JC = S // 128     # 32 token chunks of 128 (for G)
QC = NQ // 512    # 4 query chunks of 512


@with_exitstack
def _attention_kernel(ctx: ExitStack, tc: tile.TileContext):
    nc = tc.nc
    xn_d = nc.dram_tensor("xn", [C, S], FP8, kind="ExternalInput").ap()
    xt_d = nc.dram_tensor("xt", [C, S], FP16, kind="ExternalInput").ap()
    wq_d = nc.dram_tensor("wall16", [C, 512], FP16, kind="ExternalInput").ap()
    wkv_d = nc.dram_tensor("wkvr", [C, 256], F32R, kind="ExternalInput").ap()
    bo_d = nc.dram_tensor("boc", [C, 1], F32, kind="ExternalInput").ap()
    out_d = nc.dram_tensor("out_cT", [C, NQ], F32, kind="ExternalOutput").ap()

    consts = ctx.enter_context(tc.tile_pool(name="consts", bufs=1))
    big = ctx.enter_context(tc.tile_pool(name="big", bufs=1))
    pacc = ctx.enter_context(tc.tile_pool(name="pacc", bufs=1, space="PSUM"))
    psm = ctx.enter_context(tc.tile_pool(name="psm", bufs=1, space="PSUM"))
    psd = ctx.enter_context(tc.tile_pool(name="psd", bufs=2, space="PSUM"))
    psg = ctx.enter_context(tc.tile_pool(name="psg", bufs=1, space="PSUM"))
    pmm = ctx.enter_context(tc.tile_pool(name="pmm", bufs=3, space="PSUM"))

    # ---- input DMA first on both queues, few big transfers ----
    xn = big.tile([C, S], FP8)
    nc.sync.dma_start(out=xn[:], in_=xn_d)
    wall = consts.tile([C, 512], FP16)
    nc.gpsimd.dma_start(out=wall[:], in_=wq_d)
    wq = wall[:, 0:384]
    wo = wall[:, 384:512]
    xt = big.tile([C, S], FP16)
    nc.gpsimd.dma_start(out=xt[:, 0:2048], in_=xt_d[:, 0:2048])
    nc.sync.dma_start(out=xt[:, 2048:4096], in_=xt_d[:, 2048:4096])
    wkv = consts.tile([C, 256], F32R)
    nc.gpsimd.dma_start(out=wkv[:], in_=wkv_d)
    boc = consts.tile([C, 1], F32)
    nc.gpsimd.dma_start(out=boc[:], in_=bo_d)

    # ---- constants / zero-fills ----
    ones16 = consts.tile([C, 32], FP16)
    nc.gpsimd.memset(ones16[:], 1.0)
    one1 = consts.tile([1, 1], F32)
    nc.gpsimd.memset(one1[:], 1.0)
    mbd = consts.tile([C, C], FP16)
    nc.gpsimd.memset(mbd[:], 0.0)
    ksw = consts.tile([C, C], FP16)
    nc.gpsimd.memset(ksw[:], 0.0)
    dm = consts.tile([1, 4], F32)
    nc.vector.memset(dm[:], 1.0)
    wrm = consts.tile([C, 512], FP16)
    nc.vector.memset(wrm[:], 0.5)

    # preload the (single) ACT table set used later (runs during input DMA)
    nc.scalar.activation(dm[:, 1:2], dm[:, 0:1], AF.Sqrt)
    nc.scalar.activation(dm[:, 2:3], dm[:, 0:1], AF.Identity)

    # ---- PE warm-up: ~4us of junk matmuls so HAM unthrottles the clock
    # before real work arrives; result is sunk into out_d[0:1,0:2] which
    # the chunk-0 output DMA later overwrites ----
    wps = psg.tile([128, 512], F32, tag="w", name="warm")
    for i in range(6):
        nc.tensor.matmul(wps[:, :], wrm[:, 0:128], wrm[:],
                         start=(i == 0), stop=(i == 5))
    wsb = consts.tile([1, 2], F32)
    nc.vector.tensor_copy(wsb[:], wps[0:1, 0:2])
    nc.sync.dma_start(out=out_d[0:1, 0:2], in_=wsb[:])

    # ---- G = X X^T over all tokens (fp16, f32 accum) ----
    Gp = pacc.tile([C, C], F32, tag="g", name="G", padded_shape=[128, 512])
    for jc in range(JC):
        chunk = xn[:, 128 * jc:128 * jc + 128]
        nc.tensor.matmul(Gp[:, :], chunk, chunk,
                         start=(jc == 0), stop=(jc == JC - 1))

    # ---- X1 = sum_t x_t via ACT accumulate over xt halves (f32, column) ----
    xscr = big.tile([C, S], FP16)
    x1h = consts.tile([C, 2], F32)
    for t in range(2):
        nc.scalar.activation(xscr[:, 2048 * t:2048 * t + 2048],
                             xt[:, 2048 * t:2048 * t + 2048],
                             AF.Identity, accum_out=x1h[:, t:t + 1])
    x1a = consts.tile([C, 1], F32)
    nc.vector.tensor_add(x1a[:], x1h[:, 0:1], x1h[:, 1:2])
    x1c = consts.tile([C, 2], F32R)
    nc.vector.tensor_copy(x1c[:, 0:1], x1a[:])
    nc.vector.tensor_copy(x1c[:, 1:2], x1a[:])

    # ---- congruences through G (right after G stop; Gs copy leads DVE) ----
    Gs = big.tile([C, C], FP16)
    nc.vector.tensor_copy(Gs[:], Gp[:, :])
    Ts = []
    for sl in (slice(256, 384), slice(128, 256), slice(0, 128)):  # v, k, q
        Tp = psm.tile([C, C], F32, tag="t", padded_shape=[128, 512])
        nc.tensor.matmul(Tp[:, :], Gs[:], wq[:, sl], start=True, stop=True)
        Tsb = big.tile([C, C], FP16, name=f"T{sl.start}")
        nc.vector.tensor_copy(Tsb[:], Tp[:, :])
        Ts.append(Tsb)
    Tv, Tk, Tq = Ts
    Mfp = psm.tile([C, C], F32, tag="t", padded_shape=[128, 512])
    nc.tensor.matmul(Mfp[:, :], wq[:, 128:256], Tv[:], start=True, stop=True)

    # ssq rows: ones^T (w .* (G w)) = diag(w^T G w)
    prod = big.tile([C, 256], FP16)
    nc.vector.tensor_mul(prod[:, 0:128], wq[:, 0:128], Tq[:])
    nc.vector.tensor_mul(prod[:, 128:256], wq[:, 128:256], Tk[:])
    dqk = psg.tile([1, 256], F32, tag="w", padded_shape=[1, 512], name="dqk")
    nc.tensor.matmul(dqk[:, :], ones16[:, 0:1], prod[:], start=True, stop=True)
    gtmp = consts.tile([1, C], F32)
    dqs = consts.tile([1, 256], F32)
    nc.vector.tensor_copy(dqs[:], dqk[:, :])
    nc.vector.tensor_mul(gtmp[:], dqs[:, 0:128], dqs[:, 128:256])

    # ---- q projection (PE gap filler while the gamma DVE chain runs) ----
    qh = big.tile([C, NQ], FP16)
    for t in range(QC):
        pq = pmm.tile([128, 512], F32, tag="mm")
        nc.tensor.matmul(pq[:, :], wq[:, 0:128], xt[:, 512 * t:512 * t + 512],
                         start=True, stop=True)
        nc.vector.tensor_copy(qh[:, 512 * t:512 * t + 512], pq[:, :])

    # ---- g10 = Sqrt(100 * recip(p)) on columns via PE transpose ----
    gcp = psg.tile([C, 1], F32, tag="w", padded_shape=[128, 512], name="gcp")
    nc.tensor.transpose(gcp[:, :], gtmp[:], one1[:])
    pcol = consts.tile([C, 2], F32)
    nc.vector.tensor_copy(pcol[:, 0:1], gcp[:, :])
    nc.vector.reciprocal(pcol[:, 1:2], pcol[:, 0:1])
    g10 = consts.tile([C, 1], F32)
    nc.scalar.activation(g10[:], pcol[:, 1:2], AF.Sqrt, scale=100.0)
    g10 = g10[:]

    # ---- Ksum = w_k^T X1 and V1 = w_v^T X1 (f32r exact, columns) ----
    ksp = psd.tile([C, 2], F32, tag="d", padded_shape=[128, 512])
    nc.tensor.matmul(ksp[:, :], wkv[:, 0:128], x1c[:], start=True, stop=True)
    v1p = psd.tile([C, 2], F32, tag="d", padded_shape=[128, 512])
    nc.tensor.matmul(v1p[:, :], wkv[:, 128:256], x1c[:], start=True, stop=True)
    v1c = consts.tile([C, 1], F32)
    nc.vector.tensor_copy(v1c[:], v1p[:, 0:1])

    # ---- fold g10 into blockdiag M (DVE) and replicated Ksum (gpsimd) ----
    kst = consts.tile([C, 1], F32)
    nc.vector.tensor_scalar_mul(kst[:], ksp[:, 0:1], g10)
    for h in range(HEADS):
        hp = 32 * h
        nc.gpsimd.tensor_scalar_mul(ksw[hp:hp + 32, hp:hp + 32],
                                    ones16[hp:hp + 32, 0:32],
                                    kst[hp:hp + 32, 0:1])
        nc.vector.tensor_scalar_mul(mbd[hp:hp + 32, hp:hp + 32],
                                    Mfp[hp:hp + 32, hp:hp + 32],
                                    g10[hp:hp + 32, 0:1])

    # ---- main: per 512-query chunk (PE stream emitted ahead of epilogues) ----
    s1t = big.tile([C, NQ], F32)
    atv = big.tile([C, NQ], FP16)
    att = big.tile([C, NQ], FP16)
    res = big.tile([C, NQ], F32)
    INV_S = 1.0 / float(S)
    pds, pns = [], []
    for t in range(QC):
        qc = qh[:, 512 * t:512 * t + 512]
        pd = pmm.tile([128, 512], F32, tag="mm")
        nc.tensor.matmul(pd[:, :], ksw[:], qc, start=True, stop=True)
        pn = pmm.tile([128, 512], F32, tag="mm")
        nc.tensor.matmul(pn[:, :], mbd[:], qc, start=True, stop=True)
        pds.append(pd); pns.append(pn)
        # 1/den ~ 1/S - corr/S^2, pre-spread across each head's rows (DVE)
        s1c = s1t[:, 512 * t:512 * t + 512]
        nc.vector.tensor_scalar(s1c, pd[:, :], -INV_S * INV_S, INV_S,
                                op0=ALU.mult, op1=ALU.add)
        # numer + V1 (ACT, per-partition bias), then * s1 (DVE)
        nc.scalar.activation(atv[:, 512 * t:512 * t + 512], pn[:, :],
                             AF.Identity, bias=v1c[:])
        nc.vector.tensor_mul(att[:, 512 * t:512 * t + 512],
                             atv[:, 512 * t:512 * t + 512], s1c)
    for t in range(QC):
        po = pmm.tile([128, 512], F32, tag="mm")
        nc.tensor.matmul(po[:, :], wo[:], att[:, 512 * t:512 * t + 512],
                         start=True, stop=True)
        nc.scalar.activation(res[:, 512 * t:512 * t + 512], po[:, :],
                             AF.Identity, bias=boc[:])
        eng = nc.sync if t % 2 == 0 else nc.gpsimd
        eng.dma_start(out=out_d[:, 512 * t:512 * t + 512],
                      in_=res[:, 512 * t:512 * t + 512])


_CACHE = {}


def build_program():
    if "nc" not in _CACHE:
        nc = bacc.Bacc("TRN2", debug=False, target_bir_lowering=False,
                       num_devices=N_CORES)
        with tile.TileContext(nc) as tc:
            _attention_kernel(tc)
        nc.compile()
        _CACHE["nc"] = nc
    return _CACHE["nc"]


def make_in_maps(x, w_qkv, w_out, b_out):
    in_maps = []
    wall16 = np.ascontiguousarray(
        np.concatenate([w_qkv, w_out], axis=1), dtype=np.float16)
    wkvr = np.ascontiguousarray(w_qkv[:, 128:384], dtype=np.float32)
    bo = np.ascontiguousarray(b_out, dtype=np.float32).reshape(C, 1)
    for core in range(N_CORES):
        b, half = core // 2, core % 2
        xr = np.asarray(x[b], dtype=np.float16).reshape(S, C)
        # xn[p, jc*128+c] = x[jc*128+p, c] : token-chunk-major for G (fp8)
        xn = np.ascontiguousarray(xr.reshape(JC, 128, C).transpose(1, 0, 2)
                                  .reshape(128, S)).astype(_F8NP)
        # xt: channels-major, tokens rolled so this core's queries are [0,NQ)
        xt = np.ascontiguousarray(np.roll(xr, -half * NQ, axis=0).T)
        in_maps.append({
            "xn": xn, "xt": xt, "wall16": wall16, "wkvr": wkvr,
            "boc": bo,
        })
    return in_maps


def assemble_output(per_core_outs):
    out = np.zeros((4, S, C), dtype=np.float32)
    for core, r in enumerate(per_core_outs):
        b, half = core // 2, core % 2
        out[b, half * NQ:(half + 1) * NQ] = np.asarray(r, dtype=np.float32).T
    return out.reshape(4, 64, 64, C)


def kernel(x, w_qkv, w_out, b_out):
    from concourse.bass_utils import run_bass_kernel_spmd
    nc = build_program()
    in_maps = make_in_maps(x, w_qkv, w_out, b_out)
    res = run_bass_kernel_spmd(nc, in_maps, list(range(N_CORES)))
    return assemble_output([r["out_cT"] for r in res.results])


if __name__ == "__main__":
    x = np.random.randn(4, 64, 64, C).astype(np.float32)
    w_qkv = (np.random.randn(C, 384) / np.sqrt(C)).astype(np.float32)
    w_out = (np.random.randn(C, 128) / np.sqrt(128)).astype(np.float32)
    b_out = np.zeros(C, dtype=np.float32)
    out = kernel(x=x, w_qkv=w_qkv, w_out=w_out, b_out=b_out)
    print("kernel output", out.shape, out.dtype)
